# revision 21
# baseline (speedup 1.0000x reference)
"""Trainium2 Bass kernel for JetGNN (2-layer SAGEConv + global mean pool).

Single fused NEFF, src-major sharding:
  - Host: graph-aligned 25600-node slabs per core; each core owns the edges
    whose SRC lies in its slab. Edges grouped by (dst supertile of 512
    global dst slots, src%4), padded to 128-edge chunks.
  - Device per layer: For_i over 400 supertiles: dma_gather of packed fp16
    feature rows (4 nodes/row for x, 2 for h1), one-hot matmuls
    (is_equal(iota, dstv)) accumulate message partials [F, 512] in PSUM,
    scaled by per-dst 1/deg row -> fp16 partials [F, 204800] ->
    AllReduce(add) -> per-own-tile dst phase: W_l @ agg + W_r @ self,
    bias+ReLU; h1 kept resident (self path) and written packed to DRAM
    (layer-2 gather). Layer-2 idx derived on device (2*idx1 + gather half).
    Pooling on device: per-tile one-hot matmul into [16,64] graph windows.
  - Host: combine pool windows, divide by counts, final linear.
"""

import math
import os
import threading
import time

import numpy as np

import jax

try:
    jax.config.update("jax_compilation_cache_dir",
                      "/root/.cache/jax_bass_cache")
    jax.config.update("jax_persistent_cache_min_compile_time_secs", 0)
    jax.config.update("jax_persistent_cache_min_entry_size_bytes", 0)
except Exception:
    pass

import concourse.bass as bass
import concourse.tile as tile
import concourse.mybir as mybir
from concourse import bacc
from concourse.bass_utils import run_bass_kernel_spmd

N_NODES = 200000
N_GRAPHS = 4000
N_CORES = 8
IN_CH = 32
HID = 64
SLAB = 25600
NSLOT = N_CORES * SLAB          # 204800 global dst slots
ST = 512                        # dsts per supertile
NS = NSLOT // ST                # 400 supertiles
P = 128
NT = SLAB // P                  # own dst tiles (200)
GT = 16                         # graph window per tile (pooling)
PAD_DSTV = 600.0                # one-hot never matches

f32 = mybir.dt.float32
fp16 = mybir.dt.float16
i16 = mybir.dt.int16

GRP = [[0, 1, 2, 3, 4, 5, 6, 7]]


# ----------------------------------------------------------------- host prep
def _prep(edge_index, batch):
    src = np.asarray(edge_index[0], dtype=np.int32)
    dst = np.asarray(edge_index[1], dtype=np.int32)
    batch = np.asarray(batch, dtype=np.int32)

    gcnt = np.bincount(batch, minlength=N_GRAPHS)
    gends = np.cumsum(gcnt)
    targets = (np.arange(1, N_CORES) * N_NODES) // N_CORES
    gb = np.searchsorted(gends, targets)
    graph_bounds = np.concatenate([[0], gb + 1, [N_GRAPHS]])
    node_bounds = np.concatenate(
        [[0], gends[graph_bounds[1:-1] - 1], [N_NODES]]).astype(np.int64)
    ncounts = np.diff(node_bounds)
    assert ncounts.max() <= SLAB, ncounts.max()

    deg = np.bincount(dst, minlength=N_NODES)
    inv = (1.0 / np.maximum(deg, 1)).astype(np.float32)

    # node -> (core, local, slot) lookup tables
    node_core = np.repeat(np.arange(N_CORES, dtype=np.int32), ncounts)
    node_local = (np.arange(N_NODES, dtype=np.int32)
                  - np.repeat(node_bounds[:-1].astype(np.int32), ncounts))
    node_slot = node_core * SLAB + node_local

    cs = node_core[src]
    src_local = node_local[src]
    dslot = node_slot[dst]
    s_id = dslot >> 9
    din = (dslot & 511).astype(np.float32)
    par = src_local & 3

    # per-dst-slot inverse degree row [NS, 512]
    invrow = np.zeros(NSLOT, np.float16)
    for c in range(N_CORES):
        lo, hi = node_bounds[c], node_bounds[c + 1]
        invrow[c * SLAB:c * SLAB + hi - lo] = inv[lo:hi]
    invrow = invrow.reshape(NS, ST)

    # group edges by (core, supertile, parity)
    key = (((cs * NS + s_id) << 2) | par).astype(np.int16)
    order = np.argsort(key, kind="stable")
    key_s = key[order].astype(np.int32)
    nbins = N_CORES * NS * 4
    cnt = np.bincount(key_s, minlength=nbins)
    ch_par = max(1, int(math.ceil(cnt.max() / P)))      # chunks per parity
    nch = 4 * ch_par                                     # chunks / supertile
    ngath = 2
    gsz = 2 * ch_par                                     # chunks per gather
    assert gsz * P <= 1024, gsz
    slots_pad = nch * P

    starts = np.concatenate([[0], np.cumsum(cnt)[:-1]]).astype(np.int64)
    rank = np.arange(len(src), dtype=np.int64) - starts[key_s]

    k = key_s.astype(np.int64)
    core_e = k // (NS * 4)
    s_e = (k // 4) % NS
    p_e = k & 3
    slot = ((core_e * NS + s_e) * nch + p_e * ch_par) * P + rank

    total = N_CORES * NS * slots_pad
    idx1 = np.zeros(total, np.int16)
    dstv = np.full(total, PAD_DSTV, np.float16)
    sl = src_local[order]
    idx1[slot] = (sl >> 2).astype(np.int16)
    dstv[slot] = din[order].astype(np.float16)

    # idx compact wrap: [core][16, NS * slots_pad/16]
    a = idx1.reshape(N_CORES, NS, ngath, gsz * 8, 16)
    idx1w = np.ascontiguousarray(
        a.transpose(0, 4, 1, 2, 3).reshape(N_CORES, 16, -1))

    # dstv table: [core][128, NS * nch] fp16, value at (slot%128, chunk col)
    dstv = dstv.reshape(N_CORES, NS, nch, P)
    dv = np.ascontiguousarray(
        dstv.transpose(0, 3, 1, 2).reshape(N_CORES, P, -1))

    # pooling tables: g0 per (core, tile); grel [core][128, NT] f32
    g0 = np.zeros((N_CORES, NT), np.int64)
    grel = np.full((N_CORES, P, NT), 100.0, np.float32)
    for c in range(N_CORES):
        lo, hi = node_bounds[c], node_bounds[c + 1]
        bl = batch[lo:hi]
        for t in range((hi - lo + P - 1) // P):
            seg = bl[t * P:(t + 1) * P]
            g0[c, t] = seg[0]
            r = seg - seg[0]
            assert r.max() < GT, r.max()
            grel[c, :len(seg), t] = r
    return dict(node_bounds=node_bounds, graph_bounds=graph_bounds,
                ncounts=ncounts, gcnt=gcnt, ch_par=ch_par, nch=nch,
                ngath=ngath, gsz=gsz, idx1w=idx1w, dv=dv, invrow=invrow,
                g0=g0, grel=grel)


# ------------------------------------------------------------ kernel builder
def _build_nc(nch, ngath, gsz, ch_par):
    icols = NS * ngath * gsz * 8          # idx cols per 16-partition row
    ic_st = ngath * gsz * 8               # idx cols per supertile
    half = gsz * 8                        # idx cols per gather

    nc = bacc.Bacc("TRN2", target_bir_lowering=False, debug=False,
                   enable_asserts=False, num_devices=N_CORES)
    xg = nc.dram_tensor("xg", [SLAB * IN_CH // P, P], fp16,
                        kind="ExternalInput").ap()
    idx1c = nc.dram_tensor("idx1c", [16, icols], i16, kind="ExternalInput").ap()
    dvd = nc.dram_tensor("dvd", [P, NS * nch], fp16,
                         kind="ExternalInput").ap()
    invd = nc.dram_tensor("invd", [NS, ST], fp16, kind="ExternalInput").ap()
    iota_d = nc.dram_tensor("iota", [P, ST], fp16, kind="ExternalInput").ap()
    ident_d = nc.dram_tensor("ident", [P, P], fp16, kind="ExternalInput").ap()
    grel_d = nc.dram_tensor("grel", [P, NT], f32, kind="ExternalInput").ap()
    w1lT_d = nc.dram_tensor("w1lT", [IN_CH, HID], fp16,
                            kind="ExternalInput").ap()
    w1rT_d = nc.dram_tensor("w1rT", [IN_CH, HID], fp16,
                            kind="ExternalInput").ap()
    w2lT_d = nc.dram_tensor("w2lT", [HID, HID], fp16,
                            kind="ExternalInput").ap()
    w2rT_d = nc.dram_tensor("w2rT", [HID, HID], fp16,
                            kind="ExternalInput").ap()
    b1_d = nc.dram_tensor("b1c", [HID, 1], f32, kind="ExternalInput").ap()
    b2_d = nc.dram_tensor("b2c", [HID, 1], f32, kind="ExternalInput").ap()

    idx1r = nc.dram_tensor("idx1r", [P, icols], i16, kind="Internal").ap()
    h1d = nc.dram_tensor("h1d", [SLAB, HID], fp16, kind="Internal").ap()
    partb = nc.dram_tensor("partb", [HID, NSLOT], fp16, kind="Internal").ap()
    redb = nc.dram_tensor("redb", [HID, NSLOT], fp16, kind="Internal",
                          addr_space="Shared").ap()
    part1, part2 = partb[0:IN_CH, :], partb
    red1, red2 = redb[0:IN_CH, :], redb
    poolp = nc.dram_tensor("poolp", [NT, GT, HID], fp16,
                           kind="ExternalOutput").ap()

    ns_run = int(os.environ.get("K_NS", NS))
    nt_run = int(os.environ.get("K_NT", NT))

    with tile.TileContext(nc) as tc:
        with tc.tile_pool(name="res", bufs=1) as rp, \
             tc.tile_pool(name="ld", bufs=3) as ld, \
             tc.tile_pool(name="g", bufs=3) as gp, \
             tc.tile_pool(name="oh", bufs=4) as ohp, \
             tc.tile_pool(name="o", bufs=3) as op_, \
             tc.tile_pool(name="st", bufs=3) as stp, \
             tc.tile_pool(name="ps", bufs=2, space="PSUM") as ps, \
             tc.tile_pool(name="ps2", bufs=2, space="PSUM") as ps2, \
             tc.tile_pool(name="ps3", bufs=1, space="PSUM") as ps3:

            # ---- prologue: residents + idx replication to 128 partitions
            iota_sb = rp.tile([P, ST], fp16, tag="iota")
            nc.sync.dma_start(iota_sb[:], iota_d[:])
            ident_sb = rp.tile([P, P], fp16, tag="ident")
            nc.sync.dma_start(ident_sb[:], ident_d[:])
            grel_sb = rp.tile([P, NT], f32, tag="grel")
            nc.sync.dma_start(grel_sb[:], grel_d[:])
            w1lT = rp.tile([IN_CH, HID], fp16, tag="w1lT")
            nc.sync.dma_start(w1lT[:], w1lT_d[:])
            w1rT = rp.tile([IN_CH, HID], fp16, tag="w1rT")
            nc.sync.dma_start(w1rT[:], w1rT_d[:])
            w2lT = rp.tile([HID, HID], fp16, tag="w2lT")
            nc.sync.dma_start(w2lT[:], w2lT_d[:])
            w2rT = rp.tile([HID, HID], fp16, tag="w2rT")
            nc.sync.dma_start(w2rT[:], w2rT_d[:])
            b1 = rp.tile([HID, 1], f32, tag="b1")
            nc.sync.dma_start(b1[:], b1_d[:])
            b2 = rp.tile([HID, 1], f32, tag="b2")
            nc.sync.dma_start(b2[:], b2_d[:])
            h1T_res = rp.tile([HID, SLAB], fp16, tag="h1T_res")
            red_sb = rp.tile([HID, SLAB], fp16, tag="red_sb")

            for k in range(8):
                nc.sync.dma_start(idx1r[16 * k:16 * (k + 1), :], idx1c[:])
            tc.strict_bb_all_engine_barrier()

            pid = nc.sync.partition_id()

            idx3 = idx1r.rearrange("p (s c) -> p s c", c=ic_st)
            dv3 = dvd.rearrange("p (s c) -> p s c", c=nch)
            inv3 = invd.rearrange("s (o d) -> s o d", o=1)

            def supertile_loop(lay, tabv, F, partials):
                p3 = partials.rearrange("f (s d) -> f s d", d=ST)
                gslots = gsz * P
                with tc.For_i(0, ns_run) as s:
                    idx_sb = ld.tile([P, ic_st], i16, tag=f"idx{lay}")
                    nc.sync.dma_start(idx_sb[:], idx3[:, s])
                    if lay == 2:
                        idx2t = ld.tile([P, ic_st], i16, tag="idx2t")
                        for g in range(ngath):
                            nc.vector.tensor_scalar(
                                idx2t[:, g * half:(g + 1) * half],
                                idx_sb[:, g * half:(g + 1) * half],
                                2, g, op0=mybir.AluOpType.mult,
                                op1=mybir.AluOpType.add)
                        idx_sb = idx2t
                    dvh = ld.tile([P, nch], fp16, tag=f"dvh{lay}")
                    nc.sync.dma_start(dvh[:], dv3[:, s])
                    dvf = ld.tile([P, nch], f32, tag=f"dvf{lay}")
                    nc.vector.tensor_copy(dvf[:], dvh[:])
                    inv1 = ld.tile([1, ST], fp16, tag=f"inv1_{lay}")
                    nc.sync.dma_start(inv1[:], inv3[s])
                    invb = ld.tile([F, ST], fp16, tag=f"invb{lay}")
                    nc.gpsimd.partition_broadcast(invb[:], inv1[:])
                    ms = []
                    for g in range(ngath):
                        m = gp.tile([P, gsz, P], fp16, tag=f"m{lay}_{g}")
                        nc.gpsimd.dma_gather(
                            m[:], tabv,
                            idx_sb[:, g * half:(g + 1) * half],
                            gslots, gslots, P)
                        ms.append(m)
                    zp = ps.tile([F, ST], f32, tag="zp")
                    for c in range(nch):
                        par4 = min(c // ch_par, 3)
                        colblk = (par4 * IN_CH) if lay == 1 else \
                            ((par4 & 1) * HID)
                        m = ms[c // gsz]
                        oh = ohp.tile([P, ST], fp16, tag=f"oh{lay}")
                        nc.vector.tensor_scalar(
                            oh[:], iota_sb[:], dvf[:, c:c + 1], None,
                            op0=mybir.AluOpType.is_equal)
                        nc.tensor.matmul(
                            zp[:], lhsT=m[:, c % gsz, colblk:colblk + F],
                            rhs=oh[:], start=(c == 0), stop=(c == nch - 1))
                    zsb = op_.tile([F, ST], fp16, tag=f"zsb{lay}")
                    nc.vector.tensor_tensor(zsb[:], zp[:], invb[:],
                                            op=mybir.AluOpType.mult)
                    nc.sync.dma_start(p3[:, s], zsb[:])

            xr3 = xg.rearrange("(t q) (k f) -> t (q k) f", q=P // 4, k=4)

            def dst_loop(lay, red, F, wl, wr, bb, hout_res):
                redv = red_sb[0:F, :]
                nc.sync.dma_start(
                    redv,
                    red.rearrange("f (c n) -> f c n", n=SLAB)[:, pid])
                r3 = redv.rearrange("f (t d) -> f t d", d=P)
                gr3 = grel_sb.rearrange("p (t o) -> p t o", o=1)
                o3 = (hout_res.rearrange("f (t d) -> f t d", d=P)
                      if hout_res is not None else None)
                h13 = h1d.rearrange("(t d) f -> t d f", d=P)
                h1r3 = h1T_res.rearrange("f (t d) -> f t d", d=P)
                with tc.For_i(0, nt_run) as t:
                    if lay == 1:
                        xr = ld.tile([P, IN_CH], fp16, tag="xr")
                        nc.sync.dma_start(xr[:], xr3[t])
                        xTp = ps3.tile([IN_CH, P], fp16, tag="xTp")
                        nc.tensor.transpose(xTp[:], xr[:], ident_sb[:])
                        xT = ld.tile([IN_CH, P], fp16, tag="xT")
                        nc.vector.tensor_copy(xT[:], xTp[:])
                    z2 = ps2.tile([HID, P], f32, tag="z2")
                    nc.tensor.matmul(z2[:], lhsT=wl[:], rhs=r3[:, t],
                                     start=True, stop=False)
                    if lay == 1:
                        nc.tensor.matmul(z2[:], lhsT=wr[:], rhs=xT[:],
                                         start=False, stop=True)
                    else:
                        nc.tensor.matmul(z2[:], lhsT=wr[:], rhs=h1r3[:, t],
                                         start=False, stop=True)
                    hT = op_.tile([HID, P], fp16, tag=f"hT{lay}")
                    nc.scalar.activation(hT[:], z2[:],
                                         mybir.ActivationFunctionType.Relu,
                                         bias=bb[:])
                    if o3 is not None:
                        nc.vector.tensor_copy(o3[:, t], hT[:])
                    tp = ps3.tile([P, HID], fp16, tag="tp")
                    nc.tensor.transpose(tp[:], hT[:],
                                        ident_sb[0:HID, 0:HID])
                    stg = stp.tile([P, HID], fp16, tag=f"stg{lay}")
                    nc.vector.tensor_copy(stg[:], tp[:])
                    if lay == 1:
                        nc.sync.dma_start(h13[t], stg[:])
                    else:
                        ohg = stp.tile([P, GT], fp16, tag="ohg")
                        nc.vector.tensor_scalar(
                            ohg[:], iota_sb[:, 0:GT], gr3[:, t],
                            None, op0=mybir.AluOpType.is_equal)
                        pp = ps3.tile([GT, HID], f32, tag="pp")
                        nc.tensor.matmul(pp[:], lhsT=ohg[:], rhs=stg[:],
                                         start=True, stop=True)
                        ppc = stp.tile([GT, HID], fp16, tag="ppc")
                        nc.vector.tensor_copy(ppc[:], pp[:])
                        nc.sync.dma_start(poolp[t], ppc[:])

            # ---- layer 1
            supertile_loop(1, xg[:], IN_CH, part1)
            if os.environ.get("K_NOCC"):
                nc.sync.dma_start(red1, part1)
            else:
                nc.gpsimd.collective_compute(
                    kind="AllReduce", op=mybir.AluOpType.add,
                    replica_groups=GRP, ins=[part1], outs=[red1])
            dst_loop(1, red1, IN_CH, w1lT, w1rT, b1, h1T_res)

            # ---- layer 2
            h1v = h1d.rearrange("(r k) f -> r (k f)", k=2)
            supertile_loop(2, h1v, HID, part2)
            if os.environ.get("K_NOCC"):
                nc.sync.dma_start(red2, part2)
            else:
                nc.gpsimd.collective_compute(
                    kind="AllReduce", op=mybir.AluOpType.add,
                    replica_groups=GRP, ins=[part2], outs=[red2])
            dst_loop(2, red2, HID, w2lT, w2rT, b2, None)

    nc.compile()
    return nc


_NC_CACHE = {}


def kernel(x, edge_index, batch, W1_l, b1, W1_r, W2_l, b2, W2_r, W_lin,
           b_lin, _timing=None):
    x = np.asarray(x, dtype=np.float32)
    batch_np = np.asarray(batch, dtype=np.int64)

    t0 = time.time()
    # speculatively build the expected-config NEFF while prep runs
    exp_key = (12, 2, 6, 3)
    th = None
    if exp_key not in _NC_CACHE:
        def _bg():
            try:
                _NC_CACHE[exp_key] = _build_nc(*exp_key)
            except Exception:
                pass
        th = threading.Thread(target=_bg)
        th.start()
    pp = _prep(edge_index, batch_np)
    t_prep = time.time() - t0

    nch, ngath, gsz = pp["nch"], pp["ngath"], pp["gsz"]
    t0 = time.time()
    if th is not None:
        th.join()
    key = (nch, ngath, gsz, pp["ch_par"])
    if key not in _NC_CACHE:
        _NC_CACHE[key] = _build_nc(nch, ngath, gsz, pp["ch_par"])
    nc = _NC_CACHE[key]
    t_build = time.time() - t0

    nb = pp["node_bounds"]
    iota_np = np.tile(np.arange(ST, dtype=np.float16), (P, 1))
    ident_np = np.eye(P, dtype=np.float16)
    com = dict(
        iota=iota_np, ident=ident_np, invd=pp["invrow"],
        w1lT=np.ascontiguousarray(np.asarray(W1_l).T).astype(np.float16),
        w1rT=np.ascontiguousarray(np.asarray(W1_r).T).astype(np.float16),
        w2lT=np.ascontiguousarray(np.asarray(W2_l).T).astype(np.float16),
        w2rT=np.ascontiguousarray(np.asarray(W2_r).T).astype(np.float16),
        b1c=np.asarray(b1, np.float32).reshape(HID, 1),
        b2c=np.asarray(b2, np.float32).reshape(HID, 1),
    )
    in_maps = []
    for c in range(N_CORES):
        lo, hi = nb[c], nb[c + 1]
        xs = np.zeros((SLAB, IN_CH), np.float16)
        xs[:hi - lo] = x[lo:hi].astype(np.float16)
        in_maps.append(dict(
            xg=np.ascontiguousarray(xs.reshape(SLAB * IN_CH // P, P)),
            idx1c=pp["idx1w"][c], dvd=pp["dv"][c], grel=pp["grel"][c],
            **com))

    t0 = time.time()
    res = None
    for attempt in range(3):
        try:
            res = run_bass_kernel_spmd(nc, in_maps,
                                       core_ids=list(range(N_CORES)))
        except Exception:
            if attempt == 2:
                raise
            continue
        ok = True
        for r in res.results:
            pp_ = r["poolp"]
            # post-stall corruption returns NaN/Inf or silent all-zeros
            if not np.isfinite(pp_).all() or not pp_.any():
                ok = False
                break
        if ok:
            break
    t_run = time.time() - t0

    # ---- host: combine pool windows + final linear
    t0 = time.time()
    pool = np.zeros((N_GRAPHS + GT, HID), np.float32)
    for c in range(N_CORES):
        ppart = res.results[c]["poolp"].astype(np.float32)   # [NT, GT, HID]
        gidx = pp["g0"][c][:, None] + np.arange(GT)[None, :]
        np.add.at(pool, gidx.reshape(-1), ppart.reshape(-1, HID))
    pool = pool[:N_GRAPHS]
    cnt = np.maximum(pp["gcnt"], 1).astype(np.float32)
    pooled = pool / cnt[:, None]
    out = (pooled @ np.asarray(W_lin, np.float32).T
           + np.asarray(b_lin, np.float32)).astype(np.float32)
    t_host = time.time() - t0

    if _timing is not None:
        _timing.update(dtA=t_run, dtB=0.0, prep=t_prep, build=t_build,
                       host=t_host)
    return out
